# revision 13
# baseline (speedup 1.0000x reference)
"""Bass/Trainium2 kernel for nn_Attention_75471165325727 (sparse local-p attention).

reference:
    ap  = sigmoid(squeeze(tanh(enc @ W_p) @ v_p))          # [B,S]
    idx = top_k(ap, 64); mask = one_hot(idx).sum(1)        # [B,S]
    src = enc + enc * (ap*mask)[...,None] / ((ap*mask)[...,None] + 1e-7)
    score -> softmax(axis=-1) over a size-1 axis == 1.0    # W_a / decoder_out are dead
    returns (context=src*1, weights=ones[B,S,1])

Sharding: data-parallel over batch, 4 batches per core on 8 cores.
"""

import sys

sys.path.insert(0, "/opt/trn_rl_repo")

import numpy as np

import concourse.bass as bass
import concourse.mybir as mybir
import concourse.tile as tile
from concourse import bacc
from concourse.bass_utils import run_bass_kernel_spmd
from concourse.masks import make_identity

F32 = mybir.dt.float32

B, S, H = 32, 4096, 256
NCORES = 8
BL = B // NCORES          # batches per core = 4
N = BL * S                # positions per core = 16384
K = 64                    # top-k
EPS = 1e-7

GT = 512                  # gemm tile positions (psum free max for f32)
NGT = N // GT             # 32 gemm tiles per core
GT_PER_B = S // GT        # 8 chunks per batch
LOAD = 2048               # positions per load DMA (1 MB)
NLOAD = N // LOAD         # 16

# dtype used for the W_p GEMM / v_p contraction path (float32 = exact, 4 cyc/row;
# float32r = 1 cyc/row at fd>=256)
GEMM_DT = F32


def build(gemm_dt=GEMM_DT):
    nc = bacc.Bacc("TRN2", target_bir_lowering=False, debug=False,
                   num_devices=NCORES)

    enc = nc.dram_tensor("enc", [N, H], F32, kind="ExternalInput").ap()
    wp = nc.dram_tensor("wp", [H, H], F32, kind="ExternalInput").ap()
    vp = nc.dram_tensor("vp", [H, 1], F32, kind="ExternalInput").ap()
    ctx_o = nc.dram_tensor("ctx", [N, H], F32, kind="ExternalOutput").ap()
    wts_o = nc.dram_tensor("wts", [N, 1], F32, kind="ExternalOutput").ap()
    # DRAM bounce buffers for cross-partition regrouping
    cand_d = nc.dram_tensor("cand_d", [BL * GT_PER_B * K], F32).ap()
    ap_d = nc.dram_tensor("ap_d", [BL, S], F32).ap()

    with tile.TileContext(nc) as tc:
        with (
            tc.tile_pool(name="nat", bufs=NLOAD) as nat_pool,
            tc.tile_pool(name="consts", bufs=1) as consts,
            tc.tile_pool(name="encT", bufs=3) as encT_pool,
            tc.tile_pool(name="tanh", bufs=3) as tanh_pool,
            tc.tile_pool(name="small", bufs=1) as small,
            tc.tile_pool(name="pe_ps", bufs=3, space="PSUM") as pe_ps,
            tc.tile_pool(name="mm_ps", bufs=3, space="PSUM") as mm_ps,
            tc.tile_pool(name="lg_ps", bufs=2, space="PSUM") as lg_ps,
        ):
            # ---- constants ----
            ident = consts.tile([128, 128], F32)
            make_identity(nc, ident[:])
            wp_sb = consts.tile([128, 2, H], gemm_dt, tag="wp")  # [h_loc, hc, j]
            nc.sync.dma_start(
                out=wp_sb[:], in_=wp.rearrange("(c p) j -> p c j", p=128))
            vp_sb = consts.tile([128, 2], gemm_dt, tag="vp")     # [j_loc, jc]
            nc.sync.dma_start(
                out=vp_sb[:], in_=vp.rearrange("(c p) o -> p (c o)", p=128))
            ones = consts.tile([128, N // 128], F32, tag="ones")
            nc.vector.memset(ones[:], 1.0)

            ap2 = small.tile([BL * GT_PER_B, GT], F32, tag="ap2")  # [32, 512]

            enc_v = enc.rearrange("(t j p) h -> t p j h", p=128, j=LOAD // 128)
            nat = []
            for t in range(NLOAD):
                nt = nat_pool.tile([128, LOAD // 128, H], F32, tag="nat")
                nc.sync.dma_start(out=nt[:], in_=enc_v[t])
                nat.append(nt)

            # ---- pass 1: ap = sigmoid(tanh(enc @ W_p) @ v_p) ----
            for g in range(NGT):
                b, r = g // GT_PER_B, g % GT_PER_B
                ld, gg = g // (LOAD // GT), g % (LOAD // GT)
                logit_ps = lg_ps.tile([1, GT], F32, tag="logit")
                # transpose 4 position-subtiles x 2 h-chunks -> encT [h, pos]
                encT = []
                for hc in range(2):
                    e_ps = pe_ps.tile([128, GT], F32, tag="e_ps")
                    for sub in range(4):
                        j = 4 * gg + sub
                        nc.tensor.transpose(
                            out=e_ps[:, 128 * sub:128 * (sub + 1)],
                            in_=nat[ld][:, j, 128 * hc:128 * (hc + 1)],
                            identity=ident[:])
                    e_sb = encT_pool.tile([128, GT], gemm_dt, tag="e_sb")
                    if hc == 0:
                        nc.scalar.copy(out=e_sb[:], in_=e_ps[:])
                    else:
                        nc.vector.tensor_copy(e_sb[:], e_ps[:])
                    encT.append(e_sb)
                # T^T[jc] = sum_hc W_p[hc,:,jc].T @ encT[hc]
                for jc in range(2):
                    t_ps = mm_ps.tile([128, GT], F32, tag="t_ps")
                    for hc in range(2):
                        nc.tensor.matmul(
                            out=t_ps[:],
                            lhsT=wp_sb[:, hc, 128 * jc:128 * (jc + 1)],
                            rhs=encT[hc][:],
                            start=(hc == 0), stop=(hc == 1))
                    th = tanh_pool.tile([128, GT], gemm_dt, tag="th")
                    nc.scalar.activation(
                        out=th[:], in_=t_ps[:],
                        func=mybir.ActivationFunctionType.Tanh)
                    nc.tensor.matmul(
                        out=logit_ps[:],
                        lhsT=vp_sb[:, jc:jc + 1], rhs=th[:],
                        start=(jc == 0), stop=(jc == 1))
                if r == 0:
                    ap_stage = small.tile([1, S], F32, tag="ap_stage")
                nc.scalar.activation(
                    out=ap_stage[:, GT * r:GT * (r + 1)],
                    in_=logit_ps[:],
                    func=mybir.ActivationFunctionType.Sigmoid)
                if r == GT_PER_B - 1:
                    nc.gpsimd.dma_start(out=ap_d[b], in_=ap_stage[:])

            # ---- pass 2: top-64 per batch ----
            # ap2 chunk-major: row 4r+b holds batch b positions [512r, 512r+512)
            for r in range(GT_PER_B):
                nc.gpsimd.dma_start(
                    out=ap2[BL * r:BL * (r + 1), :],
                    in_=ap_d[:, GT * r:GT * (r + 1)])
            # L1: top-64 of each 512-chunk (extract 8 at a time)
            work = small.tile([BL * GT_PER_B, GT], F32, tag="work")
            cand = small.tile([BL * GT_PER_B, K], F32, tag="cand")
            nc.vector.tensor_copy(work[:], ap2[:])
            for it in range(K // 8):
                nc.vector.max(out=cand[:, 8 * it:8 * (it + 1)], in_=work[:])
                nc.vector.match_replace(
                    out=work[:], in_to_replace=cand[:, 8 * it:8 * (it + 1)],
                    in_values=work[:], imm_value=0.0)
            # regroup: cand2[b, 64r+k] = cand[4r+b, k] via DRAM bounce
            nc.gpsimd.dma_start(out=cand_d, in_=cand[:])
            cand2 = small.tile([BL, GT_PER_B * K], F32, tag="cand2")
            cand_v = cand_d.rearrange("(r b k) -> b r k", b=BL, k=K)
            nc.gpsimd.dma_start(out=cand2[:].rearrange(
                "b (r k) -> b r k", k=K), in_=cand_v)
            # L2: top-64 of the 512 candidates per batch -> sorted values v4
            v4 = small.tile([BL, K], F32, tag="v4")
            for it in range(K // 8):
                nc.vector.max(out=v4[:, 8 * it:8 * (it + 1)], in_=cand2[:])
                nc.vector.match_replace(
                    out=cand2[:], in_to_replace=v4[:, 8 * it:8 * (it + 1)],
                    in_values=cand2[:], imm_value=0.0)
            # broadcast v4 [4,64] -> v32 rows 4r+b (contiguous blocks)
            v32 = small.tile([BL * GT_PER_B, K], F32, tag="v32")
            for r in range(GT_PER_B):
                nc.gpsimd.dma_start(
                    out=v32[BL * r:BL * (r + 1), :], in_=v4[:])
            # zap the top-64 values -> ap_m = ap2 - zapped
            workz = small.tile([BL * GT_PER_B, GT], F32, tag="workz")
            nc.vector.tensor_copy(workz[:], ap2[:])
            for it in range(K // 8):
                nc.vector.match_replace(
                    out=workz[:], in_to_replace=v32[:, 8 * it:8 * (it + 1)],
                    in_values=workz[:], imm_value=0.0)
            apm = small.tile([BL * GT_PER_B, GT], F32, tag="apm")
            nc.vector.tensor_sub(out=apm[:], in0=ap2[:], in1=workz[:])
            # sc = 1 + apm/(apm+eps)
            sc = small.tile([BL * GT_PER_B, GT], F32, tag="sc")
            nc.vector.tensor_scalar_add(sc[:], apm[:], EPS)
            nc.vector.reciprocal(sc[:], sc[:])
            nc.vector.tensor_mul(out=sc[:], in0=sc[:], in1=apm[:])
            nc.vector.tensor_scalar_add(sc[:], sc[:], 1.0)
            # transpose sc to position-major: scT[p, 32c + 4r + b] =
            # scale(b, 512r + 128c + p); pos-tile (b,jj) uses col
            # 32*(jj%4) + 4*(jj//4) + b
            s_ps = mm_ps.tile([128, 128], F32, tag="t_ps")
            for c in range(4):
                nc.tensor.transpose(
                    out=s_ps[:, 32 * c:32 * (c + 1)],
                    in_=sc[:, 128 * c:128 * (c + 1)],
                    identity=ident[:32, :32])
            scT = small.tile([128, 128], F32, tag="scT")
            nc.vector.tensor_copy(scT[:], s_ps[:])

            # ---- pass 3: out = enc * sc, write back ----
            ctx_v = ctx_o.rearrange("(t j p) h -> t p j h", p=128, j=LOAD // 128)
            for t in range(NLOAD):
                for j in range(LOAD // 128):
                    pt = t * (LOAD // 128) + j      # global pos-tile 0..127
                    b, jj = pt // 32, pt % 32
                    q = 32 * (jj % 4) + 4 * (jj // 4) + b
                    nc.vector.tensor_scalar_mul(
                        nat[t][:, j, :], nat[t][:, j, :], scT[:, q:q + 1])
                nc.sync.dma_start(out=ctx_v[t], in_=nat[t][:])

            # ---- weights = ones ----
            nc.sync.dma_start(
                out=wts_o.rearrange("(p f) o -> p (f o)", p=128), in_=ones[:])

    nc.compile()
    return nc


_CACHE = {}


def _get_nc():
    if "nc" not in _CACHE:
        _CACHE["nc"] = build()
    return _CACHE["nc"]


def kernel(encoder_out, decoder_out=None, W_p=None, v_p=None, W_a=None,
           _trace=False):
    encoder_out = np.ascontiguousarray(encoder_out, dtype=np.float32)
    W_p = np.ascontiguousarray(W_p, dtype=np.float32)
    v_p = np.ascontiguousarray(v_p, dtype=np.float32)
    nc = _get_nc()
    in_maps = [
        {
            "enc": encoder_out[i * BL:(i + 1) * BL].reshape(N, H),
            "wp": W_p,
            "vp": v_p.reshape(H, 1),
        }
        for i in range(NCORES)
    ]
    res = run_bass_kernel_spmd(nc, in_maps, core_ids=list(range(NCORES)),
                               trace=_trace)
    ctx = np.concatenate(
        [res.results[i]["ctx"].reshape(BL, S, H) for i in range(NCORES)], axis=0)
    wts = np.concatenate(
        [res.results[i]["wts"].reshape(BL, S, 1) for i in range(NCORES)], axis=0)
    if _trace:
        _CACHE["last_result"] = res
    return ctx, wts


# revision 16
# speedup vs baseline: 1.5520x; 1.5520x over previous
"""Bass/Trainium2 kernel for nn_Attention_75471165325727 (sparse local-p attention).

reference:
    ap  = sigmoid(squeeze(tanh(enc @ W_p) @ v_p))          # [B,S]
    idx = top_k(ap, 64); mask = one_hot(idx).sum(1)        # [B,S]
    src = enc + enc * (ap*mask)[...,None] / ((ap*mask)[...,None] + 1e-7)
    score -> softmax(axis=-1) over a size-1 axis == 1.0    # W_a / decoder_out are dead
    returns (context=src*1, weights=ones[B,S,1])

Sharding: data-parallel over batch, 4 batches per core on 8 cores.
"""

import sys

sys.path.insert(0, "/opt/trn_rl_repo")

import numpy as np

import concourse.bass as bass
import concourse.mybir as mybir
import concourse.tile as tile
from concourse import bacc
from concourse.bass_utils import run_bass_kernel_spmd
from concourse.masks import make_identity

F32 = mybir.dt.float32

B, S, H = 32, 4096, 256
NCORES = 8
BL = B // NCORES          # batches per core = 4
N = BL * S                # positions per core = 16384
K = 64                    # top-k
EPS = 1e-7

GT = 512                  # gemm tile positions (psum free max for f32)
NGT = N // GT             # 32 gemm tiles per core
GT_PER_B = S // GT        # 8 chunks per batch
LOAD = 2048               # positions per load DMA (1 MB)
NLOAD = N // LOAD         # 16

# float32r runs the PE at 1 cyc/row (vs 4 for plain f32) for moving dims
# >= 256; storage bits are identical f32, so tiles stay f32 and matmul
# operands are bitcast views.
USE_F32R = True
R32 = mybir.dt.float32r


GDT = R32 if USE_F32R else F32


def build():
    nc = bacc.Bacc("TRN2", target_bir_lowering=False, debug=False,
                   num_devices=NCORES)

    enc = nc.dram_tensor("enc", [N, H], F32, kind="ExternalInput").ap()
    wp = nc.dram_tensor("wp", [H, H], F32, kind="ExternalInput").ap()
    vp = nc.dram_tensor("vp", [H, 1], F32, kind="ExternalInput").ap()
    ctx_o = nc.dram_tensor("ctx", [N, H], F32, kind="ExternalOutput").ap()
    wts_o = nc.dram_tensor("wts", [N, 1], F32, kind="ExternalOutput").ap()
    # DRAM bounce buffers for cross-partition regrouping
    cand_d = nc.dram_tensor("cand_d", [BL * GT_PER_B * K], F32).ap()
    ap_d = nc.dram_tensor("ap_d", [BL, S], F32).ap()

    with tile.TileContext(nc) as tc:
        with (
            tc.tile_pool(name="nat", bufs=NLOAD) as nat_pool,
            tc.tile_pool(name="consts", bufs=1) as consts,
            tc.tile_pool(name="encT", bufs=3) as encT_pool,
            tc.tile_pool(name="tanh", bufs=3) as tanh_pool,
            tc.tile_pool(name="small", bufs=1) as small,
            tc.tile_pool(name="pe_ps", bufs=3, space="PSUM") as pe_ps,
            tc.tile_pool(name="mm_ps", bufs=3, space="PSUM") as mm_ps,
            tc.tile_pool(name="lg_ps", bufs=2, space="PSUM") as lg_ps,
        ):
            # ---- constants ----
            ident = consts.tile([128, 128], F32)
            make_identity(nc, ident[:])
            wp_ld = consts.tile([128, 2, H], F32, tag="wp_ld")
            nc.sync.dma_start(
                out=wp_ld[:], in_=wp.rearrange("(c p) j -> p c j", p=128))
            vp_ld = consts.tile([128, 2], F32, tag="vp_ld")
            nc.sync.dma_start(
                out=vp_ld[:], in_=vp.rearrange("(c p) o -> p (c o)", p=128))
            wp_sb = consts.tile([128, 2, H], GDT, tag="wp")  # [h_loc, hc, j]
            nc.vector.tensor_copy(wp_sb[:], wp_ld[:])
            vp_sb = consts.tile([128, 2], GDT, tag="vp")     # [j_loc, jc]
            nc.vector.tensor_copy(vp_sb[:], vp_ld[:])
            ones = consts.tile([128, N // 128], F32, tag="ones")
            nc.vector.memset(ones[:], 1.0)

            ap2 = small.tile([BL * GT_PER_B, GT], F32, tag="ap2")  # [32, 512]

            enc_v = enc.rearrange("(t j p) h -> t p j h", p=128, j=LOAD // 128)
            nat = []
            for t in range(NLOAD):
                nt = nat_pool.tile([128, LOAD // 128, H], F32, tag="nat")
                nc.sync.dma_start(out=nt[:], in_=enc_v[t])
                nat.append(nt)

            # ---- pass 1: ap = sigmoid(tanh(enc @ W_p) @ v_p) ----
            for g in range(NGT):
                b, r = g // GT_PER_B, g % GT_PER_B
                ld, gg = g // (LOAD // GT), g % (LOAD // GT)
                logit_ps = lg_ps.tile([1, GT], F32, tag="logit")
                # transpose 4 position-subtiles x 2 h-chunks -> encT [h, pos]
                encT = []
                for hc in range(2):
                    e_ps = pe_ps.tile([128, GT], F32, tag="e_ps")
                    for sub in range(4):
                        j = 4 * gg + sub
                        nc.tensor.transpose(
                            out=e_ps[:, 128 * sub:128 * (sub + 1)],
                            in_=nat[ld][:, j, 128 * hc:128 * (hc + 1)],
                            identity=ident[:])
                    e_sb = encT_pool.tile([128, GT], GDT, tag="e_sb")
                    if hc == 0:
                        nc.scalar.copy(out=e_sb[:], in_=e_ps[:])
                    else:
                        nc.vector.tensor_copy(e_sb[:], e_ps[:])
                    encT.append(e_sb)
                # T^T[jc] = sum_hc W_p[hc,:,jc].T @ encT[hc]
                for jc in range(2):
                    t_ps = mm_ps.tile([128, GT], F32, tag="t_ps")
                    for hc in range(2):
                        nc.tensor.matmul(
                            out=t_ps[:],
                            lhsT=wp_sb[:, hc, 128 * jc:128 * (jc + 1)],
                            rhs=encT[hc][:],
                            start=(hc == 0), stop=(hc == 1))
                    th = tanh_pool.tile([128, GT], GDT, tag="th")
                    nc.scalar.activation(
                        out=th[:], in_=t_ps[:],
                        func=mybir.ActivationFunctionType.Tanh)
                    nc.tensor.matmul(
                        out=logit_ps[:],
                        lhsT=vp_sb[:, jc:jc + 1], rhs=th[:],
                        start=(jc == 0), stop=(jc == 1))
                if r == 0:
                    ap_stage = small.tile([1, S], F32, tag="ap_stage")
                nc.scalar.activation(
                    out=ap_stage[:, GT * r:GT * (r + 1)],
                    in_=logit_ps[:],
                    func=mybir.ActivationFunctionType.Sigmoid)
                if r == GT_PER_B - 1:
                    nc.gpsimd.dma_start(out=ap_d[b], in_=ap_stage[:])

            # ---- pass 2: top-64 per batch ----
            # ap2 chunk-major: row 4r+b holds batch b positions [512r, 512r+512)
            for r in range(GT_PER_B):
                nc.gpsimd.dma_start(
                    out=ap2[BL * r:BL * (r + 1), :],
                    in_=ap_d[:, GT * r:GT * (r + 1)])
            # L1: top-64 of each 512-chunk (extract 8 at a time)
            work = small.tile([BL * GT_PER_B, GT], F32, tag="work")
            cand = small.tile([BL * GT_PER_B, K], F32, tag="cand")
            nc.vector.tensor_copy(work[:], ap2[:])
            for it in range(K // 8):
                nc.vector.max(out=cand[:, 8 * it:8 * (it + 1)], in_=work[:])
                nc.vector.match_replace(
                    out=work[:], in_to_replace=cand[:, 8 * it:8 * (it + 1)],
                    in_values=work[:], imm_value=0.0)
            # regroup: cand2[b, 64r+k] = cand[4r+b, k] via DRAM bounce
            nc.gpsimd.dma_start(out=cand_d, in_=cand[:])
            cand2 = small.tile([BL, GT_PER_B * K], F32, tag="cand2")
            cand_v = cand_d.rearrange("(r b k) -> b r k", b=BL, k=K)
            nc.gpsimd.dma_start(out=cand2[:].rearrange(
                "b (r k) -> b r k", k=K), in_=cand_v)
            # L2: top-64 of the 512 candidates per batch -> sorted values v4
            v4 = small.tile([BL, K], F32, tag="v4")
            for it in range(K // 8):
                nc.vector.max(out=v4[:, 8 * it:8 * (it + 1)], in_=cand2[:])
                nc.vector.match_replace(
                    out=cand2[:], in_to_replace=v4[:, 8 * it:8 * (it + 1)],
                    in_values=cand2[:], imm_value=0.0)
            # broadcast v4 [4,64] -> v32 rows 4r+b (contiguous blocks)
            v32 = small.tile([BL * GT_PER_B, K], F32, tag="v32")
            for r in range(GT_PER_B):
                nc.gpsimd.dma_start(
                    out=v32[BL * r:BL * (r + 1), :], in_=v4[:])
            # zap the top-64 values -> ap_m = ap2 - zapped
            workz = small.tile([BL * GT_PER_B, GT], F32, tag="workz")
            nc.vector.tensor_copy(workz[:], ap2[:])
            for it in range(K // 8):
                nc.vector.match_replace(
                    out=workz[:], in_to_replace=v32[:, 8 * it:8 * (it + 1)],
                    in_values=workz[:], imm_value=0.0)
            apm = small.tile([BL * GT_PER_B, GT], F32, tag="apm")
            nc.vector.tensor_sub(out=apm[:], in0=ap2[:], in1=workz[:])
            # sc = 1 + apm/(apm+eps)
            sc = small.tile([BL * GT_PER_B, GT], F32, tag="sc")
            nc.vector.tensor_scalar_add(sc[:], apm[:], EPS)
            nc.vector.reciprocal(sc[:], sc[:])
            nc.vector.tensor_mul(out=sc[:], in0=sc[:], in1=apm[:])
            nc.vector.tensor_scalar_add(sc[:], sc[:], 1.0)
            # transpose sc to position-major: scT[p, 32c + 4r + b] =
            # scale(b, 512r + 128c + p); pos-tile (b,jj) uses col
            # 32*(jj%4) + 4*(jj//4) + b
            s_ps = mm_ps.tile([128, 128], F32, tag="t_ps")
            for c in range(4):
                nc.tensor.transpose(
                    out=s_ps[:, 32 * c:32 * (c + 1)],
                    in_=sc[:, 128 * c:128 * (c + 1)],
                    identity=ident[:32, :32])
            scT = small.tile([128, 128], F32, tag="scT")
            nc.vector.tensor_copy(scT[:], s_ps[:])

            # ---- pass 3: out = enc * sc, write back ----
            ctx_v = ctx_o.rearrange("(t j p) h -> t p j h", p=128, j=LOAD // 128)
            for t in range(NLOAD):
                for j in range(LOAD // 128):
                    pt = t * (LOAD // 128) + j      # global pos-tile 0..127
                    b, jj = pt // 32, pt % 32
                    q = 32 * (jj % 4) + 4 * (jj // 4) + b
                    nc.vector.tensor_scalar_mul(
                        nat[t][:, j, :], nat[t][:, j, :], scT[:, q:q + 1])
                nc.sync.dma_start(out=ctx_v[t], in_=nat[t][:])

            # ---- weights = ones ----
            nc.sync.dma_start(
                out=wts_o.rearrange("(p f) o -> p (f o)", p=128), in_=ones[:])

    nc.compile()
    return nc


_CACHE = {}


def _get_nc():
    if "nc" not in _CACHE:
        _CACHE["nc"] = build()
    return _CACHE["nc"]


def kernel(encoder_out, decoder_out=None, W_p=None, v_p=None, W_a=None,
           _trace=False):
    encoder_out = np.ascontiguousarray(encoder_out, dtype=np.float32)
    W_p = np.ascontiguousarray(W_p, dtype=np.float32)
    v_p = np.ascontiguousarray(v_p, dtype=np.float32)
    nc = _get_nc()
    in_maps = [
        {
            "enc": encoder_out[i * BL:(i + 1) * BL].reshape(N, H),
            "wp": W_p,
            "vp": v_p.reshape(H, 1),
        }
        for i in range(NCORES)
    ]
    res = run_bass_kernel_spmd(nc, in_maps, core_ids=list(range(NCORES)),
                               trace=_trace)
    ctx = np.concatenate(
        [res.results[i]["ctx"].reshape(BL, S, H) for i in range(NCORES)], axis=0)
    wts = np.concatenate(
        [res.results[i]["wts"].reshape(BL, S, 1) for i in range(NCORES)], axis=0)
    if _trace:
        _CACHE["last_result"] = res
    return ctx, wts
